# revision 1
# baseline (speedup 1.0000x reference)
"""Axial MHA (temporal-then-social) Trainium2 kernel, one batch sample per core.

Hardware constraint discovered on TRN2: matmul operands (fmap+weights) must
START at SBUF partition 0 (row-strip tile_position packing crashes at runtime);
PSUM output col-strips (tile_position[1] in {0,32,64,96}) work fine.

DRAM layouts (per core, batch sample b):
  qT, kT, vT [256, 8192] f32r-bits track-major cols (col = n*128 + l)
  w*T [256, 256] f32r [d_in, e_out] (= torch_w.T); q-proj weights pre-scaled 1/sqrt(dk)
  outT [256, 8192] f32 position-major cols (col = l*64 + n)

Phase A (8 chunks x 8 tracks):
  f32r projections tq/tk -> bf16 staging [128, CHW] -> DMA-shift to head-split
  tqh/tkh [32, H*CHW]; s_k + fused (s_wq@t_wd) s_q projections -> position-major
  bf16 residents; tv natural bf16; per-track temporal attention:
  scores S^T (8 serial K=32 MMs, full-M) -> exp [128,1024] -> ones-matmul
  denominators + PV (col-strip packed) -> reciprocal * PV -> t_att (f32r).
Phase B (8 chunks x 16 positions):
  re-read v (position gather); sv natural [64, pos*256] bf16 (M=64 projs);
  DMA-shift s_q/s_k chunk slabs to sqh/skh [32, H*1024]; per-2-position social
  attention at 64 lanes; final projection + bias -> outT.
"""
import sys
import numpy as np
from contextlib import ExitStack

import os as _os_early
for _p in ("/opt/trn_rl_repo", "/root/.axon_site/_ro/trn_rl_repo"):
    if _os_early.path.isdir(_p) and _p not in sys.path:
        sys.path.insert(0, _p)
import concourse.bass as bass
import concourse.tile as tile
from concourse import mybir, bacc

F32 = mybir.dt.float32
F32R = mybir.dt.float32r
BF16 = mybir.dt.bfloat16
EXP = mybir.ActivationFunctionType.Exp

D, L, NTR, H, DK, T = 256, 128, 64, 8, 32, 8192
CH_A, TR_PER_CH = 8, 8
CH_B, POS_PER_CH = 8, 16
CHW = 1024
NS = CHW // 512

W_NAMES = ["t_wq", "t_wk", "t_wv", "sq_w", "s_wk", "s_wv", "s_wd"]

_tile_ctr = [0]


def _mk(pool, shape, dtype, tag):
    _tile_ctr[0] += 1
    return pool.tile(shape, dtype, tag=tag, name=f"{tag}_{_tile_ctr[0]}")


def build_nc(attn_dt=BF16, proj_dt=F32R, phases="AB"):
    nc = bacc.Bacc("TRN2")
    qT = nc.declare_dram_parameter("qT", [D, T], proj_dt, isOutput=False)
    kT = nc.declare_dram_parameter("kT", [D, T], proj_dt, isOutput=False)
    vT = nc.declare_dram_parameter("vT", [D, T], proj_dt, isOutput=False)
    w_dram = {n: nc.declare_dram_parameter(n, [D, D], proj_dt, isOutput=False)
              for n in W_NAMES}
    sq_b = nc.declare_dram_parameter("sq_b", [D], F32, isOutput=False)
    s_bd = nc.declare_dram_parameter("s_bd", [D], F32, isOutput=False)
    outT = nc.declare_dram_parameter("outT", [D, T], F32, isOutput=True)

    with tile.TileContext(nc) as tc, ExitStack() as ctx:
        wpool = ctx.enter_context(tc.tile_pool(name="wpool", bufs=1))
        res_pool = ctx.enter_context(tc.tile_pool(name="res", bufs=1))

        w_sb = {}
        for n in W_NAMES:
            w_sb[n] = []
            for kt in range(2):
                wt = _mk(wpool, [128, D], proj_dt, f"w_{n}_{kt}")
                nc.sync.dma_start(wt[:], w_dram[n].ap()[kt * 128:(kt + 1) * 128, :])
                w_sb[n].append(wt)
        sq_b_sb = _mk(wpool, [128, 2], F32, "sq_b")
        nc.sync.dma_start(sq_b_sb[:], sq_b.ap().rearrange("(e p) -> p e", p=128))
        s_bd_sb = _mk(wpool, [128, 2], F32, "s_bd")
        nc.sync.dma_start(s_bd_sb[:], s_bd.ap().rearrange("(e p) -> p e", p=128))
        ones_sb = _mk(wpool, [128, 32], attn_dt, "ones")
        nc.vector.memset(ones_sb[:], 1.0)

        # residents: position-major (col = l*64 + n), heads stacked on partitions
        s_kT_res = [_mk(res_pool, [128, T], attn_dt, f"skT{g}") for g in range(2)]
        s_qT_res = [_mk(res_pool, [128, T], attn_dt, f"sqT{g}") for g in range(2)]
        skT_pm = [t.rearrange("p (l n) -> p n l", n=NTR) for t in s_kT_res]
        sqT_pm = [t.rearrange("p (l n) -> p n l", n=NTR) for t in s_qT_res]

        pspool = ctx.enter_context(tc.tile_pool(name="pp", bufs=2, space="PSUM"))

        def proj_T(wname, xin, writeback):
            """out^T[e, t-chunk]; writeback(e, ns, psum[128,512])."""
            for e in range(2):
                ps = [_mk(pspool, [128, 512], F32, "pj") for _ in range(NS)]
                for kt in range(2):
                    for j in range(NS):
                        nc.tensor.matmul(
                            ps[j][:],
                            w_sb[wname][kt][:, e * 128:(e + 1) * 128],
                            xin[kt][:, j * 512:(j + 1) * 512],
                            start=(kt == 0), stop=(kt == 1))
                for j in range(NS):
                    writeback(e, j, ps[j])

        # ================= PHASE A =================
        if "A" not in phases:
            for g in range(2):
                nc.vector.memset(s_kT_res[g][:], 0.0)
                nc.vector.memset(s_qT_res[g][:], 0.0)
        if "A" in phases:
         with ExitStack() as actx:
            a_in = actx.enter_context(tc.tile_pool(name="a_in", bufs=1))
            a_st = actx.enter_context(tc.tile_pool(name="a_st", bufs=2))
            a_h = actx.enter_context(tc.tile_pool(name="a_h", bufs=2))
            a_ta = actx.enter_context(tc.tile_pool(name="a_ta", bufs=2))
            a_tv = actx.enter_context(tc.tile_pool(name="a_tv", bufs=2))
            a_exp = actx.enter_context(tc.tile_pool(name="a_exp", bufs=3))
            a_r = actx.enter_context(tc.tile_pool(name="a_r", bufs=3))

            for c in range(CH_A):
                c0 = c * CHW
                q_sb, k_sb, v_sb = [], [], []
                for kt in range(2):
                    qt = _mk(a_in, [128, CHW], proj_dt, f"q{kt}")
                    nc.sync.dma_start(qt[:], qT.ap()[kt * 128:(kt + 1) * 128, c0:c0 + CHW])
                    q_sb.append(qt)
                    ktile = _mk(a_in, [128, CHW], proj_dt, f"k{kt}")
                    nc.sync.dma_start(ktile[:], kT.ap()[kt * 128:(kt + 1) * 128, c0:c0 + CHW])
                    k_sb.append(ktile)
                    vt = _mk(a_in, [128, CHW], proj_dt, f"v{kt}")
                    nc.sync.dma_start(vt[:], vT.ap()[kt * 128:(kt + 1) * 128, c0:c0 + CHW])
                    v_sb.append(vt)

                # tq/tk: proj -> bf16 staging -> shift-DMA to head-split layout
                tqh = _mk(a_h, [32, H * CHW], attn_dt, "tqh")
                tkh = _mk(a_h, [32, H * CHW], attn_dt, "tkh")
                for wname, xin, dsth in (("t_wq", q_sb, tqh), ("t_wk", k_sb, tkh)):
                    for e in range(2):
                        ps = [_mk(pspool, [128, 512], F32, "pj") for _ in range(NS)]
                        for kt in range(2):
                            for j in range(NS):
                                nc.tensor.matmul(
                                    ps[j][:],
                                    w_sb[wname][kt][:, e * 128:(e + 1) * 128],
                                    xin[kt][:, j * 512:(j + 1) * 512],
                                    start=(kt == 0), stop=(kt == 1))
                        st = _mk(a_st, [128, CHW], attn_dt, "st")
                        for j in range(NS):
                            nc.vector.tensor_copy(st[:, j * 512:(j + 1) * 512], ps[j][:])
                        for hh in range(4):
                            h = 4 * e + hh
                            nc.scalar.dma_start(
                                dsth[0:32, h * CHW:(h + 1) * CHW],
                                st[32 * hh:32 * hh + 32, :])

                # s_k projection -> position-major resident
                proj_T("s_wk", k_sb,
                       lambda e, ns, ps: nc.scalar.copy(
                           skT_pm[e][:, 8 * c + 4 * ns: 8 * c + 4 * ns + 4, :],
                           ps[:].rearrange("p (n l) -> p n l", l=L)))

                # temporal V natural (track tl at cols [tl*256, (tl+1)*256)) + attention
                tv_all = _mk(a_tv, [128, TR_PER_CH * 256], attn_dt, "tv")
                t_att = [_mk(a_ta, [128, CHW], proj_dt, f"ta{g}") for g in range(2)]
                for tl in range(TR_PER_CH):
                    ts0 = tl * 128
                    psum = _mk(pspool, [128, 256], F32, "pj")
                    for kt in range(2):
                        nc.tensor.matmul(psum[:],
                                         v_sb[kt][:, tl * 128:(tl + 1) * 128],
                                         w_sb["t_wv"][kt][:, :],
                                         start=(kt == 0), stop=(kt == 1))
                    nc.scalar.copy(tv_all[:, tl * 256:(tl + 1) * 256], psum[:])

                    psumS = _mk(pspool, [128, 1024], F32, "S")
                    for h in range(H):
                        nc.tensor.matmul(
                            psumS[:, h * 128:(h + 1) * 128],
                            tkh[0:32, h * CHW + ts0: h * CHW + ts0 + 128],
                            tqh[0:32, h * CHW + ts0: h * CHW + ts0 + 128])
                    expP = _mk(a_exp, [128, 1024], attn_dt, "expP")
                    nc.scalar.activation(expP[:], psumS[:], EXP)
                    psumOD = _mk(pspool, [128, 512], F32, "od")
                    psumO = psumOD[:, 0:256]
                    psumD = psumOD[:, 256:512]
                    for g in range(2):
                        for cc in range(4):
                            h = 4 * g + cc
                            nc.tensor.matmul(
                                psumD[32 * cc:32 * (cc + 1), g * 128:(g + 1) * 128],
                                ones_sb[:, :],
                                expP[:, h * 128:(h + 1) * 128],
                                tile_position=(0, 32 * cc))
                            nc.tensor.matmul(
                                psumO[32 * cc:32 * (cc + 1), g * 128:(g + 1) * 128],
                                tv_all[:, tl * 256 + h * 32: tl * 256 + (h + 1) * 32],
                                expP[:, h * 128:(h + 1) * 128],
                                tile_position=(0, 32 * cc))
                    rec = _mk(a_r, [128, 256], F32, "rec")
                    nc.vector.reciprocal(rec[:], psumD[:])
                    for g in range(2):
                        nc.vector.tensor_mul(
                            t_att[g][:, ts0:ts0 + 128],
                            psumO[:, g * 128:(g + 1) * 128],
                            rec[:, g * 128:(g + 1) * 128])

                # fused social-Q projection (+bias) -> position-major resident
                proj_T("sq_w", t_att,
                       lambda e, ns, ps: nc.vector.tensor_scalar_add(
                           sqT_pm[e][:, 8 * c + 4 * ns: 8 * c + 4 * ns + 4, :],
                           ps[:].rearrange("p (n l) -> p n l", l=L),
                           sq_b_sb[:, e:e + 1]))

        # ================= PHASE B =================
        if "B" not in phases:
            st = _mk(res_pool, [128, 256], F32, "zz")
            nc.vector.memset(st[:], 0.0)
            for e in range(2):
                for jj in range(32):
                    nc.sync.dma_start(
                        outT.ap()[e * 128:(e + 1) * 128, jj * 256:(jj + 1) * 256], st[:])
        if "B" in phases:
         with ExitStack() as bctx:
            b_in = bctx.enter_context(tc.tile_pool(name="b_in", bufs=2))
            b_h = bctx.enter_context(tc.tile_pool(name="b_h", bufs=1))
            b_sv = bctx.enter_context(tc.tile_pool(name="b_sv", bufs=2))
            b_att = bctx.enter_context(tc.tile_pool(name="b_att", bufs=2))
            b_out = bctx.enter_context(tc.tile_pool(name="b_out", bufs=2))
            b_exp = bctx.enter_context(tc.tile_pool(name="b_exp", bufs=3))
            b_r = bctx.enter_context(tc.tile_pool(name="b_r", bufs=3))

            for d in range(CH_B):
                l0 = d * POS_PER_CH
                # head-split q/k slabs for this chunk (position-major source)
                sqh = _mk(b_h, [32, H * POS_PER_CH * 64], attn_dt, "sqh")
                skh = _mk(b_h, [32, H * POS_PER_CH * 64], attn_dt, "skh")
                for h in range(H):
                    g, hh = h // 4, h % 4
                    nc.scalar.dma_start(
                        skh[0:32, h * 1024:(h + 1) * 1024],
                        s_kT_res[g][32 * hh:32 * hh + 32, l0 * 64: l0 * 64 + 1024])
                    nc.scalar.dma_start(
                        sqh[0:32, h * 1024:(h + 1) * 1024],
                        s_qT_res[g][32 * hh:32 * hh + 32, l0 * 64: l0 * 64 + 1024])

                # v gather (local col = n*16 + p)
                vp_sb = []
                for kt in range(2):
                    vt = _mk(b_in, [128, CHW], proj_dt, f"vp{kt}")
                    src = vT.ap()[kt * 128:(kt + 1) * 128, :].rearrange(
                        "p (n l) -> p n l", n=NTR)[:, :, l0:l0 + POS_PER_CH]
                    nc.sync.dma_start(vt[:], src)
                    vp_sb.append(vt)

                sv_all = _mk(b_sv, [64, POS_PER_CH * 256], attn_dt, "sv")
                s_att = [_mk(b_att, [128, CHW], proj_dt, f"sa{g}") for g in range(2)]
                for p2 in range(POS_PER_CH // 2):
                    for pl in range(2):
                        p = 2 * p2 + pl
                        psum = _mk(pspool, [64, 256], F32, "pj")
                        for kt in range(2):
                            lhsT = vp_sb[kt].rearrange(
                                "p (n t) -> p t n", n=NTR)[:, p, :]
                            nc.tensor.matmul(psum[:], lhsT, w_sb["s_wv"][kt][:, :],
                                             start=(kt == 0), stop=(kt == 1))
                        nc.scalar.copy(sv_all[0:64, p * 256:(p + 1) * 256], psum[:])

                    # scores for 2 positions: [64, pl*512 + h*64 + q]
                    psumS = _mk(pspool, [64, 1024], F32, "S")
                    for pl in range(2):
                        p = 2 * p2 + pl
                        for h in range(H):
                            nc.tensor.matmul(
                                psumS[0:64, pl * 512 + 64 * h: pl * 512 + 64 * h + 64],
                                skh[0:32, h * 1024 + p * 64: h * 1024 + p * 64 + 64],
                                sqh[0:32, h * 1024 + p * 64: h * 1024 + p * 64 + 64])
                    expP = _mk(b_exp, [64, 1024], attn_dt, "expP2")
                    nc.scalar.activation(expP[:], psumS[:], EXP)

                    psumOD = _mk(pspool, [128, 512], F32, "od")
                    psumO = psumOD[:, 0:256]
                    psumD = psumOD[:, 256:512]
                    for pl in range(2):
                        p = 2 * p2 + pl
                        for h in range(H):
                            rr, cc = h // 4, h % 4
                            rhs = expP[0:64, pl * 512 + 64 * h: pl * 512 + 64 * h + 64]
                            nc.tensor.matmul(
                                psumD[32 * cc:32 * cc + 32,
                                      pl * 128 + 64 * rr: pl * 128 + 64 * rr + 64],
                                ones_sb[0:64, :], rhs,
                                tile_position=(0, 32 * cc))
                            nc.tensor.matmul(
                                psumO[32 * cc:32 * cc + 32,
                                      pl * 128 + 64 * rr: pl * 128 + 64 * rr + 64],
                                sv_all[0:64, p * 256 + h * 32: p * 256 + (h + 1) * 32],
                                rhs,
                                tile_position=(0, 32 * cc))
                    rec = _mk(b_r, [128, 256], F32, "rec2")
                    nc.vector.reciprocal(rec[:], psumD[:])
                    for g in range(2):
                        po = psumO.rearrange("p (pl b) -> p pl b", pl=2)[:, :, 64 * g:64 * g + 64]
                        rc = rec.rearrange("p (pl b) -> p pl b", pl=2)[:, :, 64 * g:64 * g + 64]
                        dst = s_att[g][:, p2 * 128:(p2 + 1) * 128].rearrange(
                            "p (pl b) -> p pl b", pl=2)
                        nc.vector.tensor_mul(dst, po, rc)

                outst = [_mk(b_out, [128, CHW], F32, f"os{e}") for e in range(2)]
                proj_T("s_wd", s_att,
                       lambda e, ns, ps: nc.vector.tensor_scalar_add(
                           outst[e][:, ns * 512:(ns + 1) * 512],
                           ps[:], s_bd_sb[:, e:e + 1]))
                for e in range(2):
                    nc.scalar.dma_start(outT.ap()[e * 128:(e + 1) * 128,
                                                  d * CHW:(d + 1) * CHW], outst[e][:])

    nc.compile()
    return nc


# ---------------- host-side packing ----------------
def pack_weights(t_wq, t_wk, t_wv, t_wd, t_bd, s_wq, s_wk, s_wv, s_wd, s_bd):
    s = 1.0 / np.sqrt(DK)
    t_wq_s = (t_wq * s).astype(np.float32)
    s_wq_s = (s_wq * s).astype(np.float32)
    sq_w = s_wq_s @ t_wd
    sq_bv = s_wq_s @ t_bd
    c = np.ascontiguousarray
    return {
        "t_wq": c(t_wq_s.T), "t_wk": c(t_wk.T), "t_wv": c(t_wv.T),
        "sq_w": c(sq_w.T.astype(np.float32)),
        "s_wk": c(s_wk.T), "s_wv": c(s_wv.T), "s_wd": c(s_wd.T),
        "sq_b": c(sq_bv.astype(np.float32)), "s_bd": c(s_bd),
    }


def pack_core_inputs(q_b, k_b, v_b, weights):
    qTh = np.ascontiguousarray(q_b.transpose(2, 1, 0).reshape(D, T))   # [D, n*128+l]
    kTh = np.ascontiguousarray(k_b.transpose(2, 1, 0).reshape(D, T))
    vTh = np.ascontiguousarray(v_b.transpose(2, 1, 0).reshape(D, T))
    return {"qT": qTh, "kT": kTh, "vT": vTh, **weights}


def unpack_core_output(outT_np):
    # outT [D, l*64+n] -> [L, NTR, D]
    return np.ascontiguousarray(outT_np.reshape(D, L, NTR).transpose(1, 2, 0))


# ---------------- numpy reference (per core) ----------------
def ref_core(q_b, k_b, v_b, t_wq, t_wk, t_wv, t_wd, t_bd, s_wq, s_wk, s_wv, s_wd, s_bd):
    def lin(x, w, b=None):
        y = x @ w.T
        return y if b is None else y + b

    def sdpa(q, k, v):
        s = (q @ np.swapaxes(k, -1, -2)) / np.sqrt(q.shape[-1])
        s = s - s.max(-1, keepdims=True)
        p = np.exp(s)
        p = p / p.sum(-1, keepdims=True)
        return p @ v

    Lq, N, Dm = q_b.shape
    qt = np.swapaxes(q_b, 0, 1)
    kt = np.swapaxes(k_b, 0, 1)
    vt = np.swapaxes(v_b, 0, 1)
    qt = lin(qt, t_wq).reshape(N, Lq, H, DK).transpose(0, 2, 1, 3)
    kt = lin(kt, t_wk).reshape(N, Lq, H, DK).transpose(0, 2, 1, 3)
    vt = lin(vt, t_wv).reshape(N, Lq, H, DK).transpose(0, 2, 1, 3)
    x = sdpa(qt, kt, vt).transpose(0, 2, 1, 3).reshape(N, Lq, Dm)
    t_out = lin(np.swapaxes(x, 0, 1), t_wd, t_bd)

    qs = lin(t_out, s_wq).reshape(Lq, N, H, DK).transpose(0, 2, 1, 3)
    ks = lin(k_b, s_wk).reshape(Lq, N, H, DK).transpose(0, 2, 1, 3)
    vs = lin(v_b, s_wv).reshape(Lq, N, H, DK).transpose(0, 2, 1, 3)
    x = sdpa(qs, ks, vs).transpose(0, 2, 1, 3).reshape(Lq, N, Dm)
    return lin(x, s_wd, s_bd)


# ====================== harness entry point ======================
import os as _os

for _p in ("/opt/trn_rl_repo", _os.path.expanduser("~/.axon_site/_ro/trn_rl_repo")):
    if _os.path.isdir(_p) and _p not in sys.path:
        sys.path.insert(0, _p)

_NC_CACHE = {}


def _get_nc():
    if "nc" not in _NC_CACHE:
        _NC_CACHE["nc"] = build_nc()
    return _NC_CACHE["nc"]


def _get_executor():
    """Cached jitted SPMD executable over the 8 cores (same lowering path as
    bass_utils.run_bass_kernel_spmd's axon redirect, reused across calls)."""
    if "exec" in _NC_CACHE:
        return _NC_CACHE["exec"]
    import jax
    from jax.sharding import Mesh, PartitionSpec, NamedSharding
    from jax.experimental.shard_map import shard_map
    from concourse.bass2jax import (_bass_exec_p, partition_id_tensor,
                                    install_neuronx_cc_hook)

    install_neuronx_cc_hook()
    nc = _get_nc()
    partition_name = nc.partition_id_tensor.name if nc.partition_id_tensor else None
    in_names, out_names, out_avals, zero_outs = [], [], [], []
    for alloc in nc.m.functions[0].allocations:
        if not isinstance(alloc, mybir.MemoryLocationSet):
            continue
        name = alloc.memorylocations[0].name
        if alloc.kind == "ExternalInput" and name != partition_name:
            in_names.append(name)
        elif alloc.kind == "ExternalOutput":
            out_names.append(name)
            shape = tuple(alloc.tensor_shape)
            dtype = mybir.dt.np(alloc.dtype)
            out_avals.append(jax.core.ShapedArray(shape, dtype))
            zero_outs.append(np.zeros(shape, dtype))
    all_names = list(in_names) + out_names
    if partition_name:
        all_names.append(partition_name)

    def _body(*args):
        operands = list(args)
        if partition_name is not None:
            operands.append(partition_id_tensor())
        return tuple(_bass_exec_p.bind(
            *operands, out_avals=tuple(out_avals), in_names=tuple(all_names),
            out_names=tuple(out_names), lowering_input_output_aliases=(),
            sim_require_finite=True, sim_require_nnan=True, nc=nc))

    devices = None
    for plat in ("axon", "neuron", None):
        try:
            devices = (jax.devices(plat) if plat else jax.devices())[:8]
            if len(devices) >= 8:
                break
        except RuntimeError:
            continue
    assert devices is not None and len(devices) >= 8, "need 8 neuron cores"
    mesh = Mesh(np.asarray(devices), ("core",))
    sh = NamedSharding(mesh, PartitionSpec("core"))
    nspec = len(in_names) + len(out_names)
    sharded = jax.jit(shard_map(_body, mesh=mesh,
                                in_specs=(PartitionSpec("core"),) * nspec,
                                out_specs=(PartitionSpec("core"),) * len(out_names),
                                check_rep=False), keep_unused=True)
    zeros_d = [jax.device_put(np.zeros((8 * z.shape[0], *z.shape[1:]), z.dtype), sh)
               for z in zero_outs]
    _NC_CACHE["exec"] = (sharded, in_names, out_names, sh, zeros_d, jax)
    return _NC_CACHE["exec"]


def kernel(query, key, value,
           t_wq, t_wk, t_wv, t_wd, t_bd,
           s_wq, s_wk, s_wv, s_wd, s_bd):
    """Full-input axial MHA on 8 NeuronCores (batch sharded, 1 sample/core)."""
    query = np.asarray(query, dtype=np.float32)
    key = np.asarray(key, dtype=np.float32)
    value = np.asarray(value, dtype=np.float32)
    w = pack_weights(np.asarray(t_wq, np.float32), np.asarray(t_wk, np.float32),
                     np.asarray(t_wv, np.float32), np.asarray(t_wd, np.float32),
                     np.asarray(t_bd, np.float32), np.asarray(s_wq, np.float32),
                     np.asarray(s_wk, np.float32), np.asarray(s_wv, np.float32),
                     np.asarray(s_wd, np.float32), np.asarray(s_bd, np.float32))
    B = query.shape[0]
    assert B == 8, f"expected batch 8, got {B}"
    in_maps = [pack_core_inputs(query[b], key[b], value[b], w) for b in range(B)]
    sharded, in_names, out_names, sh, zeros_d, jax = _get_executor()
    args_d = []
    for nm in in_names:
        cat = np.concatenate([np.asarray(in_maps[c][nm]) for c in range(B)], axis=0)
        args_d.append(jax.device_put(cat, sh))
    outs = sharded(*args_d, *zeros_d)
    oi = out_names.index("outT")
    full = np.asarray(outs[oi]).reshape(8, D, T)
    return np.stack([unpack_core_output(full[b]) for b in range(B)])

